# revision 3
# baseline (speedup 1.0000x reference)
"""Fused 2D-RoPE multi-head attention block for Trainium2, SPMD over 8 NeuronCores.

Problem: x[2,4,24,24,1024] -> qkv proj -> 16-head attention with 2-axis RoPE
-> out proj.  Data-parallel: the fused (b t) dim has 8 sequences; one
sequence (S=576 tokens, D=1024) per core.

Device-side layout choices (everything picked so no on-device transposes are
needed):
  - x is fed pre-transposed per core: xT [D, S].
  - q,k are produced in [e, s] layout (head-dim on partitions) by using the
    (host-pre-transposed) weight as the stationary operand.
  - Within each head, q/k weight rows are host-permuted to even-pairs-first
    order so the RoPE rotate-half pair swap becomes a contiguous
    32-partition block swap (plain DMAs; strided-partition DMA is broken).
  - v is produced in natural [s, e] layout (x as stationary operand), padded
    with a ones-column per head (65-wide slots) so the softmax denominator
    falls out of the same matmul that computes att@v.
  - Attention is computed as scoresT[sk, sq] = k_ropedT-stationary x
    q_ropedT, exp on ScalarE (no max subtraction: scores ~ N(0,1), exp is
    safe), then oT[dh, sq] = v_aug-stationary x E, which leaves oT in
    exactly the [d, s] layout the output projection needs as its stationary
    operand.
  - All matmuls run as float32r (full PE rate at moving-dim >= 256).

Outputs of the 8 cores are gathered and reshaped on the host; b_out is added
on the host (it is all-zeros in the reference inputs anyway).
"""

import numpy as np
from contextlib import ExitStack

B, T, HH, WW, D = 2, 4, 24, 24, 1024
NH, HD = 16, 64
S = HH * WW            # 576
BT = B * T             # 8
NCORES = 8
P = 128
SQH = 288              # half of S; moving-dim per scores/att@v matmul
NKD = D // P           # 8 contraction tiles over D
S_TILES = [(0, 128), (128, 128), (256, 128), (384, 128), (512, 64)]
VSLOT = HD + 1         # 65: per-head v columns + ones column

_CACHE: dict = {}


def _rope_tables():
    """cos/sin tables in the permuted (evens-first) [128, S] block layout."""
    half = HD // 4     # 16
    inv = (1.0 / (10000.0 ** (np.arange(half, dtype=np.float32) / np.float32(half)))).astype(np.float32)
    th = np.arange(HH, dtype=np.float32)[:, None] * inv[None, :]          # [H, 16]
    tw = np.arange(WW, dtype=np.float32)[:, None] * inv[None, :]          # [W, 16]
    cosg = np.concatenate([
        np.broadcast_to(np.cos(th)[:, None, :], (HH, WW, half)),
        np.broadcast_to(np.cos(tw)[None, :, :], (HH, WW, half))], axis=-1).reshape(S, 2 * half)
    sing = np.concatenate([
        np.broadcast_to(np.sin(th)[:, None, :], (HH, WW, half)),
        np.broadcast_to(np.sin(tw)[None, :, :], (HH, WW, half))], axis=-1).reshape(S, 2 * half)
    cosb = np.concatenate([cosg, cosg], axis=1).T          # [64, S]
    sinb = np.concatenate([-sing, sing], axis=1).T         # [64, S]
    cosb = np.ascontiguousarray(np.vstack([cosb, cosb]).astype(np.float32))   # [128, S]
    sinb = np.ascontiguousarray(np.vstack([sinb, sinb]).astype(np.float32))
    return cosb, sinb


def _head_perm():
    """Permutation of w_qkv q/k rows: within each head, evens then odds."""
    perm64 = np.concatenate([np.arange(0, HD, 2), np.arange(1, HD, 2)])
    return (np.arange(NH)[:, None] * HD + perm64[None, :]).reshape(-1)     # [1024]


def _build_nc():
    import concourse.bacc as bacc
    import concourse.mybir as mybir
    from concourse.tile import TileContext

    f32 = mybir.dt.float32
    f32r = mybir.dt.float32r
    AF = mybir.ActivationFunctionType

    nc = bacc.Bacc("TRN2", target_bir_lowering=False, debug=False)
    xT_d = nc.dram_tensor("xT", [D, S], f32r, kind="ExternalInput").ap()
    wqk_d = nc.dram_tensor("wqkT", [D, 2 * D], f32r, kind="ExternalInput").ap()
    wv_d = nc.dram_tensor("wvT", [D, D], f32r, kind="ExternalInput").ap()
    wo_d = nc.dram_tensor("woT", [D, D], f32r, kind="ExternalInput").ap()
    cos_d = nc.dram_tensor("cosb", [P, S], f32, kind="ExternalInput").ap()
    sin_d = nc.dram_tensor("sinb", [P, S], f32, kind="ExternalInput").ap()
    ones_d = nc.dram_tensor("onesc", [P, 5 * NH], f32r, kind="ExternalInput").ap()
    out_d = nc.dram_tensor("out", [S, D], f32, kind="ExternalOutput").ap()

    with TileContext(nc) as tc, ExitStack() as ctx:
        const = ctx.enter_context(tc.tile_pool(name="const", bufs=1))
        wqkp = ctx.enter_context(tc.tile_pool(name="wqkp", bufs=8))
        wvp = ctx.enter_context(tc.tile_pool(name="wvp", bufs=4))
        wop = ctx.enter_context(tc.tile_pool(name="wop", bufs=4))
        rawp = ctx.enter_context(tc.tile_pool(name="rawp", bufs=3))
        swp = ctx.enter_context(tc.tile_pool(name="swp", bufs=3))
        m2p = ctx.enter_context(tc.tile_pool(name="m2p", bufs=3))
        ep = ctx.enter_context(tc.tile_pool(name="ep", bufs=12))
        r1p = ctx.enter_context(tc.tile_pool(name="r1p", bufs=4))
        rrp = ctx.enter_context(tc.tile_pool(name="rrp", bufs=4))
        stp = ctx.enter_context(tc.tile_pool(name="stp", bufs=4))
        psum = ctx.enter_context(tc.tile_pool(name="psum", bufs=8, space="PSUM"))

        # ---- resident tensors
        xt = const.tile([P, NKD * S], f32r, name="xt")
        for i in range(NKD):
            nc.sync.dma_start(xt[:, i * S:(i + 1) * S], xT_d[i * P:(i + 1) * P, :])
        cosb = const.tile([P, S], f32, name="cosb_t")
        nc.sync.dma_start(cosb[:, :], cos_d[:, :])
        sinb = const.tile([P, S], f32, name="sinb_t")
        nc.sync.dma_start(sinb[:, :], sin_d[:, :])
        roped = const.tile([P, 2 * NH * S], f32r, name="roped")    # 16 e-tiles (q then k)
        va = const.tile([P, 5 * NH * VSLOT], f32r, name="va")      # v, 65-wide head slots
        vav = va.rearrange("p (j h c) -> p j h c", j=5, c=VSLOT)
        nc.sync.dma_start(va.rearrange("p (g c) -> p g c", c=VSLOT)[:, :, HD:HD + 1],
                          ones_d[:, :])
        oT = const.tile([P, NKD * S], f32r, name="oT")

        # ---- q,k projection (+ RoPE) : qkT[e, s] = wqkT.T @ xT
        for et in range(16):
            ps0 = psum.tile([P, SQH], f32, tag="ps", name="ps_qk0")
            ps1 = psum.tile([P, SQH], f32, tag="ps", name="ps_qk1")
            for kt in range(NKD):
                wt = wqkp.tile([P, P], f32r, name="wt")
                nc.sync.dma_start(wt[:, :], wqk_d[kt * P:(kt + 1) * P, et * P:(et + 1) * P])
                w_r = wt[:, :]
                nc.tensor.matmul(ps0[:, :], w_r, xt[:, kt * S:kt * S + SQH],
                                 start=(kt == 0), stop=(kt == NKD - 1))
                nc.tensor.matmul(ps1[:, :], w_r, xt[:, kt * S + SQH:kt * S + S],
                                 start=(kt == 0), stop=(kt == NKD - 1))
            raw = rawp.tile([P, S], f32, name="raw")
            nc.scalar.activation(raw[:, 0:SQH], ps0[:, :], AF.Copy)
            nc.scalar.activation(raw[:, SQH:S], ps1[:, :], AF.Copy)
            sw = swp.tile([P, S], f32, name="sw")
            for b0 in range(0, P, 64):
                nc.sync.dma_start(sw[b0:b0 + 32, :], raw[b0 + 32:b0 + 64, :])
                nc.sync.dma_start(sw[b0 + 32:b0 + 64, :], raw[b0:b0 + 32, :])
            rsl = roped[:, et * S:(et + 1) * S]
            m2 = m2p.tile([P, S], f32, name="m2")
            nc.vector.tensor_mul(m2[:, :], sw[:, :], sinb[:, :])
            nc.vector.tensor_mul(rsl, raw[:, :], cosb[:, :])
            nc.vector.tensor_add(rsl, rsl, m2[:, :])

        # ---- v projection : v[s, e] = xT-tiles.T @ wvT, written into 65-wide slots
        for nhf in range(2):
            psv = [psum.tile([P, 512], f32, tag="ps", name=f"ps_v{st}") for st in range(5)]
            for kt in range(NKD):
                wvt = wvp.tile([P, 512], f32r, name="wvt")
                nc.sync.dma_start(wvt[:, :], wv_d[kt * P:(kt + 1) * P, nhf * 512:(nhf + 1) * 512])
                wv_r = wvt[:, :]
                for st, (s0, sl) in enumerate(S_TILES):
                    nc.tensor.matmul(psv[st][0:sl, :],
                                     xt[:, kt * S + s0:kt * S + s0 + sl], wv_r,
                                     start=(kt == 0), stop=(kt == NKD - 1))
            for st, (s0, sl) in enumerate(S_TILES):
                dst = vav[0:sl, st:st + 1, nhf * 8:(nhf + 1) * 8, 0:HD]
                src = psv[st][0:sl, :].rearrange("p (h c) -> p h c", c=HD)
                nc.scalar.activation(dst, src, AF.Copy)

        # ---- attention, per (head, sq-half)
        for h in range(16):
            ti = h // 2
            off = (h % 2) * 64
            qb = ti * S
            kb = (8 + ti) * S
            for hf in range(2):
                qsl = roped[off:off + 64, qb + hf * SQH:qb + (hf + 1) * SQH]
                Es = []
                for j, (k0, kl) in enumerate(S_TILES):
                    ps_s = psum.tile([P, SQH], f32, tag="ps", name="ps_s")
                    nc.tensor.matmul(ps_s[0:kl, :],
                                     roped[off:off + 64, kb + k0:kb + k0 + kl],
                                     qsl, start=True, stop=True)
                    E = ep.tile([P, SQH], f32r, name="E")
                    nc.scalar.activation(E[0:kl, :], ps_s[0:kl, :], AF.Exp, scale=0.125)
                    Es.append(E)
                ps_o = psum.tile([P, SQH], f32, tag="ps", name="ps_o")
                for j, (k0, kl) in enumerate(S_TILES):
                    nc.tensor.matmul(ps_o[0:VSLOT, :],
                                     vav[0:kl, j:j + 1, h:h + 1, :],
                                     Es[j][0:kl, :],
                                     start=(j == 0), stop=(j == 4))
                r1 = r1p.tile([1, SQH], f32, name="r1")
                nc.vector.reciprocal(r1[:, :], ps_o[HD:HD + 1, :])
                rr = rrp.tile([64, SQH], f32, name="rr")
                nc.gpsimd.partition_broadcast(rr[:, :], r1[:, :])
                nc.vector.tensor_mul(oT[off:off + 64, ti * S + hf * SQH:ti * S + (hf + 1) * SQH],
                                     ps_o[0:HD, :], rr[:, :])

        # ---- output projection : out[s, e] = oT-tiles.T @ woT
        for nhf in range(2):
            pso = [psum.tile([P, 512], f32, tag="ps", name=f"ps_o{st}") for st in range(5)]
            for kt in range(NKD):
                wot = wop.tile([P, 512], f32r, name="wot")
                nc.sync.dma_start(wot[:, :], wo_d[kt * P:(kt + 1) * P, nhf * 512:(nhf + 1) * 512])
                wo_r = wot[:, :]
                for st, (s0, sl) in enumerate(S_TILES):
                    nc.tensor.matmul(pso[st][0:sl, :],
                                     oT[:, kt * S + s0:kt * S + s0 + sl], wo_r,
                                     start=(kt == 0), stop=(kt == NKD - 1))
            for st, (s0, sl) in enumerate(S_TILES):
                stg = stp.tile([P, 512], f32, name="stg")
                nc.scalar.activation(stg[0:sl, :], pso[st][0:sl, :], AF.Copy)
                nc.sync.dma_start(out_d[s0:s0 + sl, nhf * 512:(nhf + 1) * 512], stg[0:sl, :])

    nc.compile()
    return nc


def _prep_inputs(x, w_qkv, w_out):
    x = np.asarray(x, dtype=np.float32)
    w_qkv = np.asarray(w_qkv, dtype=np.float32)
    w_out = np.asarray(w_out, dtype=np.float32)
    xr = x.reshape(BT, S, D)
    perm = _head_perm()
    wq = w_qkv[0:D][perm]
    wk = w_qkv[D:2 * D][perm]
    wqkT = np.ascontiguousarray(np.concatenate([wq, wk], axis=0).T)
    wvT = np.ascontiguousarray(w_qkv[2 * D:3 * D].T)
    woT = np.ascontiguousarray(w_out.T)
    cosb, sinb = _rope_tables()
    in_maps = []
    for i in range(NCORES):
        in_maps.append({
            "xT": np.ascontiguousarray(xr[i].T),
            "wqkT": wqkT, "wvT": wvT, "woT": woT,
            "cosb": cosb, "sinb": sinb,
            "onesc": np.ones((P, 5 * NH), dtype=np.float32),
        })
    return in_maps


def get_nc():
    if "nc" not in _CACHE:
        _CACHE["nc"] = _build_nc()
    return _CACHE["nc"]


def kernel(x, w_qkv, w_out, b_out):
    from concourse import bass_utils
    nc = get_nc()
    in_maps = _prep_inputs(x, w_qkv, w_out)
    res = bass_utils.run_bass_kernel_spmd(nc, in_maps, core_ids=list(range(NCORES)))
    out = np.stack([res.results[i]["out"] for i in range(NCORES)], axis=0)
    out = out + np.asarray(b_out, dtype=np.float32)[None, None, :]
    return np.ascontiguousarray(out.reshape(B, T, HH, WW, D).astype(np.float32))
